# revision 1
# baseline (speedup 1.0000x reference)
"""Block-circulant matmul kernel for 8 Trainium2 NeuronCores.

Reference op (per token row x of shape (4096,)):
    y[i*256+n] = sum_j sum_m x[j*256+m] * d[j*256+m] * W[i, j, (m-n) % 256] + bias
i.e. y = (x*d) @ M + bias with M[(j,m),(i,n)] = W[i,j,(m-n)%256].

Sharding: data-parallel over the batch (8192 tokens -> 1024 per core);
W/bias-derived matrices are replicated.

This file is self-contained: shapes are hardcoded, no sibling imports.
"""
import os
import sys

for _p in ("/root/.axon_site", "/root/.axon_site/_ro/trn_rl_repo", "/root/.axon_site/_ro/pypackages"):
    if _p not in sys.path:
        sys.path.append(_p)

import numpy as np

import concourse.bass as bass
import concourse.tile as tile
from concourse import bacc, mybir
from concourse import bass_utils

N_CORES = 8
B = 8192
D = 4096
BS = 256
K = 16            # blocks per side
NT = B // N_CORES  # tokens per core (1024)

F32 = mybir.dt.float32
F32R = mybir.dt.float32r

LAST_EXEC_NS = None

_DENSE_CACHE = {}


def _build_dense_nc():
    """out yT (4096, NT) = M^T-style product: yT[o, t] = sum_k m[k, o] * xT[k, t] + bias[o]."""
    nc = bacc.Bacc("TRN2", target_bir_lowering=False, debug=False)
    xT = nc.dram_tensor("xT", [D, NT], F32R, kind="ExternalInput").ap()
    m = nc.dram_tensor("m", [D, D], F32R, kind="ExternalInput").ap()
    bias = nc.dram_tensor("bias", [D], F32, kind="ExternalInput").ap()
    yT = nc.dram_tensor("yT", [D, NT], F32, kind="ExternalOutput").ap()

    KT = D // 128          # 32 contraction tiles
    OB = D // 128          # 32 output blocks
    TH = NT // 512         # 2 token halves

    with tile.TileContext(nc) as tc:
        with (
            tc.tile_pool(name="xpool", bufs=KT) as xpool,
            tc.tile_pool(name="mpool", bufs=3) as mpool,
            tc.tile_pool(name="bpool", bufs=1) as bpool,
            tc.tile_pool(name="opool", bufs=6) as opool,
            tc.tile_pool(name="psum", bufs=8, space="PSUM") as psum_pool,
        ):
            bias_t = bpool.tile([128, OB], F32)
            nc.sync.dma_start(bias_t[:], bias.rearrange("(c p) -> p c", p=128))

            xts = []
            for kt in range(KT):
                xt = xpool.tile([128, NT], F32R, tag="x")
                nc.sync.dma_start(xt[:], xT[kt * 128:(kt + 1) * 128, :])
                xts.append(xt)

            for ob in range(OB):
                mt = mpool.tile([128, KT * 128], F32R, tag="m")
                # SBUF layout: partition = within-K-tile row, free = (kt, out-col)
                nc.sync.dma_start(
                    mt[:].rearrange("p (t o) -> p t o", t=KT),
                    m[:, ob * 128:(ob + 1) * 128].rearrange("(t p) o -> p t o", p=128),
                )
                for th in range(TH):
                    ps = psum_pool.tile([128, 512], F32)
                    for kt in range(KT):
                        nc.tensor.matmul(
                            ps[:],
                            mt[:, kt * 128:(kt + 1) * 128],
                            xts[kt][:, th * 512:(th + 1) * 512],
                            start=(kt == 0),
                            stop=(kt == KT - 1),
                        )
                    osb = opool.tile([128, 512], F32, tag="o")
                    nc.vector.tensor_scalar_add(osb[:], ps[:], bias_t[:, ob:ob + 1])
                    nc.sync.dma_start(yT[ob * 128:(ob + 1) * 128, th * 512:(th + 1) * 512], osb[:])
    nc.compile()
    return nc


def _dense_host_prep(x, W, d_bernoulli, bias):
    xd = (x.astype(np.float32) * d_bernoulli.astype(np.float32)[None, :])
    xT = np.ascontiguousarray(xd.T)                      # (D, B)
    midx = (np.arange(BS)[:, None] - np.arange(BS)[None, :]) % BS
    M = np.empty((D, D), dtype=np.float32)
    for i in range(K):
        for j in range(K):
            M[j * BS:(j + 1) * BS, i * BS:(i + 1) * BS] = W[i, j][midx]
    return xT, M


def kernel(x, W, d_bernoulli, bias):
    global LAST_EXEC_NS
    x = np.asarray(x, dtype=np.float32)
    W = np.asarray(W, dtype=np.float32)
    d_bernoulli = np.asarray(d_bernoulli, dtype=np.float32)
    bias = np.asarray(bias, dtype=np.float32)

    xT, M = _dense_host_prep(x, W, d_bernoulli, bias)

    if "nc" not in _DENSE_CACHE:
        _DENSE_CACHE["nc"] = _build_dense_nc()
    nc = _DENSE_CACHE["nc"]

    in_maps = []
    for c in range(N_CORES):
        in_maps.append({
            "xT": np.ascontiguousarray(xT[:, c * NT:(c + 1) * NT]),
            "m": M,
            "bias": bias,
        })

    trace = bool(os.environ.get("BASS_TRACE"))
    res = bass_utils.run_bass_kernel_spmd(
        nc, in_maps, list(range(N_CORES)), trace=trace,
        tmpdir=os.environ.get("BASS_TRACE_DIR") or None,
    )
    LAST_EXEC_NS = res.exec_time_ns

    out = np.empty((B, D), dtype=np.float32)
    for c in range(N_CORES):
        out[c * NT:(c + 1) * NT, :] = res.results[c]["yT"].T
    return out


# revision 3
# speedup vs baseline: 1.4893x; 1.4893x over previous
"""Block-circulant matmul kernel for 8 Trainium2 NeuronCores.

Reference op (per token row x of shape (4096,)):
    y = (x*d) @ M + bias,  M[(j,m),(i,n)] = W[i,j,(m-n)%256]  (circulant blocks)

Implementation (default "fft"): real-DFT factorization executed in three
matmul stages per core, data-parallel over the batch (1024 tokens/core):
  stage1: per input block j, project onto the 256-col real DFT basis (fp32r)
  stage2: per frequency-group G (4 pair-slots), one 128x128 block-diag mix (bf16)
  stage3: per output block i, inverse real DFT basis + bias (bf16)
Between stages, two SBUF->SBUF partition-shuffle DMA passes regroup the
data (frequency-major <-> block-major).  ~7.6x fewer FLOPs than dense.

"dense" fallback: y^T = M^T x^T as a plain fp32r matmul.

Self-contained: shapes hardcoded; no sibling imports.
"""
import os
import sys

for _p in ("/root/.axon_site", "/root/.axon_site/_ro/trn_rl_repo", "/root/.axon_site/_ro/pypackages"):
    if _p not in sys.path:
        sys.path.append(_p)

import numpy as np
import ml_dtypes

import concourse.bass as bass
import concourse.tile as tile
from concourse import bacc, mybir
from concourse import bass_utils

N_CORES = 8
B = 8192
D = 4096
BS = 256
K = 16             # blocks per side
NSLOT = BS // 2    # 128 frequency pair-slots
NT = B // N_CORES  # tokens per core (1024)
TC = 512           # token chunk
NCH = NT // TC     # chunks (2)

F32 = mybir.dt.float32
F32R = mybir.dt.float32r
BF16 = mybir.dt.bfloat16
BF16_NP = ml_dtypes.bfloat16

LAST_EXEC_NS = None
_CACHE = {}


# ---------------------------------------------------------------- host math

def _canonical_mats(W):
    m = np.arange(BS)
    T = np.zeros((BS, BS), np.float64)
    T[:, 0] = 1.0
    T[:, 1] = (-1.0) ** m
    for f in range(1, NSLOT):
        T[:, 2 * f] = np.cos(2 * np.pi * f * m / BS)
        T[:, 2 * f + 1] = np.sin(2 * np.pi * f * m / BS)

    Wf = np.fft.fft(W.astype(np.float64), axis=-1)
    p = Wf.real
    q = -Wf.imag

    jj = np.arange(K)
    M_slot = np.zeros((NSLOT, 2 * K, 2 * K), np.float64)
    for f in range(1, NSLOT):
        pf, qf = p[:, :, f], q[:, :, f]          # [i, j]
        M_slot[f][np.ix_(2 * jj, 2 * jj)] = pf.T
        M_slot[f][np.ix_(2 * jj + 1, 2 * jj)] = qf.T
        M_slot[f][np.ix_(2 * jj, 2 * jj + 1)] = qf.T
        M_slot[f][np.ix_(2 * jj + 1, 2 * jj + 1)] = -pf.T
    M_slot[0][np.ix_(2 * jj, 2 * jj)] = p[:, :, 0].T
    M_slot[0][np.ix_(2 * jj + 1, 2 * jj + 1)] = p[:, :, NSLOT].T

    n = np.arange(BS)
    R = np.zeros((BS, BS), np.float64)
    R[0, :] = 1.0 / BS
    R[1, :] = ((-1.0) ** n) / BS
    for f in range(1, NSLOT):
        R[2 * f, :] = 2.0 / BS * np.cos(2 * np.pi * f * n / BS)
        R[2 * f + 1, :] = -2.0 / BS * np.sin(2 * np.pi * f * n / BS)
    return T, M_slot, R


def _fft_host_mats(W, bias):
    T, M_slot, R = _canonical_mats(W)
    p_idx = np.arange(128)

    # tb_dram (128, 4*128): [p, (mt*2+pb)*128+col] = T[mt*128+p, colmap(pb,col)]
    tb = np.zeros((128, 512), np.float32)
    for pb in range(2):
        slot = 64 * pb + 4 * (p_idx // 8) + (p_idx % 8) // 2
        c = p_idx % 2
        cols = 2 * slot + c                       # canonical comp per device col
        for mt in range(2):
            tb[:, (mt * 2 + pb) * 128:(mt * 2 + pb + 1) * 128] = \
                T[mt * 128:(mt + 1) * 128, :][:, cols]

    # mix_dram (128, 32*128) bf16: [row, G*128+col]
    mix = np.zeros((128, 32 * 128), np.float64)
    kk = np.arange(K)
    for G in range(32):
        MG = np.zeros((128, 128), np.float64)
        for r in range(4):
            blk = M_slot[4 * G + r]
            for c in range(2):
                for cp in range(2):
                    MG[np.ix_(16 * (2 * r + c) + kk, 16 * (2 * r + cp) + kk)] = \
                        blk[np.ix_(2 * kk + c, 2 * kk + cp)]
        mix[:, G * 128:(G + 1) * 128] = MG

    # r_dram (128, 4*128): [p, (kt*2+nb)*128+col] = R[rowmap(kt,p), nb*128+col]
    rd = np.zeros((128, 512), np.float64)
    for kt in range(2):
        gl = p_idx // 8
        q = (p_idx % 8) // 2
        c = p_idx % 2
        rows = 2 * (64 * kt + 4 * gl + q) + c
        for nb in range(2):
            rd[:, (kt * 2 + nb) * 128:(kt * 2 + nb + 1) * 128] = \
                R[rows, :][:, nb * 128:(nb + 1) * 128]

    return (tb.astype(np.float32),
            mix.astype(BF16_NP),
            rd.astype(BF16_NP),
            bias.astype(BF16_NP).reshape(1, D))


# ---------------------------------------------------------------- fft kernel

def _build_fft_nc():
    nc = bacc.Bacc("TRN2", target_bir_lowering=False, debug=False)
    xT = nc.dram_tensor("xT", [D, NT], F32R, kind="ExternalInput").ap()
    tb_d = nc.dram_tensor("tb", [128, 512], F32R, kind="ExternalInput").ap()
    mix_d = nc.dram_tensor("mix", [128, 32 * 128], BF16, kind="ExternalInput").ap()
    r_d = nc.dram_tensor("rmat", [128, 512], BF16, kind="ExternalInput").ap()
    brow_d = nc.dram_tensor("brow", [1, D], BF16, kind="ExternalInput").ap()
    yT = nc.dram_tensor("yT", [D, NT], F32, kind="ExternalOutput").ap()

    ec = [0]

    def evac(dst, src):
        # alternate PSUM->SBUF evacuation between DVE and ACT
        if ec[0] % 2 == 0:
            nc.vector.tensor_copy(dst, src)
        else:
            nc.scalar.copy(dst, src)
        ec[0] += 1

    with tile.TileContext(nc) as tc:
        with (
            tc.tile_pool(name="consts", bufs=1) as consts,
            tc.tile_pool(name="xpool", bufs=4) as xpool,
            tc.tile_pool(name="upool", bufs=3) as upool,
            tc.tile_pool(name="u2pool", bufs=1) as u2pool,
            tc.tile_pool(name="v2pool", bufs=8) as v2pool,
            tc.tile_pool(name="vpool", bufs=1) as vpool,
            tc.tile_pool(name="ypool", bufs=6) as ypool,
            tc.tile_pool(name="psA", bufs=3, space="PSUM") as psA,
            tc.tile_pool(name="psB", bufs=2, space="PSUM") as psB,
            tc.tile_pool(name="psC", bufs=3, space="PSUM") as psC,
        ):
            tb_sb = consts.tile([128, 512], F32R)
            nc.sync.dma_start(tb_sb[:], tb_d[:])
            mix_sb = consts.tile([128, 32 * 128], BF16)
            nc.sync.dma_start(mix_sb[:], mix_d[:])
            r_sb = consts.tile([128, 512], BF16)
            nc.sync.dma_start(r_sb[:], r_d[:])
            brow_sb = consts.tile([1, D], BF16)
            nc.sync.dma_start(brow_sb[:], brow_d[:])
            ones_sb = consts.tile([1, TC], BF16)
            nc.vector.memset(ones_sb[:], 1.0)

            for ch in range(NCH):
                t0 = ch * TC
                # ---- stage 1: per block j, real-DFT projection (fp32r) ----
                u_sb = []
                for pb in range(2):
                    u_pb = upool.tile([128, K * TC], BF16, tag="u")
                    u_sb.append(u_pb)
                for j in range(K):
                    x_t = xpool.tile([128, 2 * TC], F32R, tag="x")
                    nc.sync.dma_start(
                        x_t[:].rearrange("p (mt t) -> p mt t", mt=2),
                        xT[j * BS:(j + 1) * BS, t0:t0 + TC]
                        .rearrange("(mt p) t -> p mt t", p=128),
                    )
                    for pb in range(2):
                        ps1 = psA.tile([128, TC], F32, tag="ps1")
                        for mt in range(2):
                            nc.tensor.matmul(
                                ps1[:],
                                tb_sb[:, (mt * 2 + pb) * 128:(mt * 2 + pb + 1) * 128],
                                x_t[:, mt * TC:(mt + 1) * TC],
                                start=(mt == 0), stop=(mt == 1),
                            )
                        evac(u_sb[pb][:, j * TC:(j + 1) * TC], ps1[:])

                # ---- shuffle1: (pb, gl) -> U2 group tiles ----
                u2_sb = u2pool.tile([128, 32 * TC], BF16, tag="u2")
                for pb in range(2):
                    for gl in range(16):
                        G = 16 * pb + gl
                        nc.sync.dma_start(
                            u2_sb[:, G * TC:(G + 1) * TC],
                            u_sb[pb][8 * gl:8 * gl + 8, :],
                        )

                # ---- stage 2 + shuffle2 ----
                v_sb = vpool.tile([128, 32 * TC], BF16, tag="v")
                for G in range(32):
                    ps2 = psB.tile([128, TC], F32, tag="ps2")
                    nc.tensor.matmul(
                        ps2[:],
                        mix_sb[:, G * 128:(G + 1) * 128],
                        u2_sb[:, G * TC:(G + 1) * TC],
                        start=True, stop=True,
                    )
                    v2_t = v2pool.tile([128, TC], BF16, tag="v2")
                    evac(v2_t[:], ps2[:])
                    kt, gl = G // 16, G % 16
                    nc.sync.dma_start(
                        v_sb[8 * gl:8 * gl + 8, kt * 16 * TC:(kt + 1) * 16 * TC],
                        v2_t[:],
                    )

                # ---- stage 3: per output block i, inverse basis + bias ----
                for i in range(K):
                    for nb in range(2):
                        ps3 = psC.tile([128, TC], F32, tag="ps3")
                        for kt in range(2):
                            nc.tensor.matmul(
                                ps3[:],
                                r_sb[:, (kt * 2 + nb) * 128:(kt * 2 + nb + 1) * 128],
                                v_sb[:, (kt * 16 + i) * TC:(kt * 16 + i + 1) * TC],
                                start=(kt == 0), stop=False,
                            )
                        ob = i * 2 + nb
                        nc.tensor.matmul(
                            ps3[:],
                            brow_sb[0:1, ob * 128:(ob + 1) * 128],
                            ones_sb[0:1, :],
                            start=False, stop=True,
                        )
                        y_t = ypool.tile([128, TC], F32, tag="y")
                        evac(y_t[:], ps3[:])
                        nc.sync.dma_start(
                            yT[ob * 128:(ob + 1) * 128, t0:t0 + TC], y_t[:])
    nc.compile()
    return nc


# ---------------------------------------------------------------- dense kernel

def _build_dense_nc():
    nc = bacc.Bacc("TRN2", target_bir_lowering=False, debug=False)
    xT = nc.dram_tensor("xT", [D, NT], F32R, kind="ExternalInput").ap()
    m = nc.dram_tensor("m", [D, D], F32R, kind="ExternalInput").ap()
    bias = nc.dram_tensor("bias", [D], F32, kind="ExternalInput").ap()
    yT = nc.dram_tensor("yT", [D, NT], F32, kind="ExternalOutput").ap()

    KT = D // 128
    OB = D // 128
    TH = NT // 512

    with tile.TileContext(nc) as tc:
        with (
            tc.tile_pool(name="xpool", bufs=KT) as xpool,
            tc.tile_pool(name="mpool", bufs=3) as mpool,
            tc.tile_pool(name="bpool", bufs=1) as bpool,
            tc.tile_pool(name="opool", bufs=6) as opool,
            tc.tile_pool(name="psum", bufs=8, space="PSUM") as psum_pool,
        ):
            bias_t = bpool.tile([128, OB], F32)
            nc.sync.dma_start(bias_t[:], bias.rearrange("(c p) -> p c", p=128))

            xts = []
            for kt in range(KT):
                xt = xpool.tile([128, NT], F32R, tag="x")
                nc.sync.dma_start(xt[:], xT[kt * 128:(kt + 1) * 128, :])
                xts.append(xt)

            for ob in range(OB):
                mt = mpool.tile([128, KT * 128], F32R, tag="m")
                nc.sync.dma_start(
                    mt[:].rearrange("p (t o) -> p t o", t=KT),
                    m[:, ob * 128:(ob + 1) * 128].rearrange("(t p) o -> p t o", p=128),
                )
                for th in range(TH):
                    ps = psum_pool.tile([128, 512], F32)
                    for kt in range(KT):
                        nc.tensor.matmul(
                            ps[:],
                            mt[:, kt * 128:(kt + 1) * 128],
                            xts[kt][:, th * 512:(th + 1) * 512],
                            start=(kt == 0), stop=(kt == KT - 1),
                        )
                    osb = opool.tile([128, 512], F32, tag="o")
                    nc.vector.tensor_scalar_add(osb[:], ps[:], bias_t[:, ob:ob + 1])
                    nc.sync.dma_start(
                        yT[ob * 128:(ob + 1) * 128, th * 512:(th + 1) * 512], osb[:])
    nc.compile()
    return nc


# ---------------------------------------------------------------- entry point

def _run(nc, in_maps):
    global LAST_EXEC_NS
    trace = bool(os.environ.get("BASS_TRACE"))
    res = bass_utils.run_bass_kernel_spmd(
        nc, in_maps, list(range(N_CORES)), trace=trace,
        tmpdir=os.environ.get("BASS_TRACE_DIR") or None,
    )
    LAST_EXEC_NS = res.exec_time_ns
    return res


def kernel(x, W, d_bernoulli, bias):
    x = np.asarray(x, dtype=np.float32)
    W = np.asarray(W, dtype=np.float32)
    d_bernoulli = np.asarray(d_bernoulli, dtype=np.float32)
    bias = np.asarray(bias, dtype=np.float32)

    impl = os.environ.get("KERNEL_IMPL", "fft")
    xT = np.ascontiguousarray((x * d_bernoulli[None, :]).T)

    if impl == "dense":
        if "dense" not in _CACHE:
            _CACHE["dense"] = _build_dense_nc()
        midx = (np.arange(BS)[:, None] - np.arange(BS)[None, :]) % BS
        M = np.empty((D, D), dtype=np.float32)
        for i in range(K):
            for j in range(K):
                M[j * BS:(j + 1) * BS, i * BS:(i + 1) * BS] = W[i, j][midx]
        in_maps = [
            {"xT": np.ascontiguousarray(xT[:, c * NT:(c + 1) * NT]),
             "m": M, "bias": bias}
            for c in range(N_CORES)
        ]
        res = _run(_CACHE["dense"], in_maps)
    else:
        if "fft" not in _CACHE:
            _CACHE["fft"] = _build_fft_nc()
        tb, mix, rd, brow = _fft_host_mats(W, bias)
        in_maps = [
            {"xT": np.ascontiguousarray(xT[:, c * NT:(c + 1) * NT]),
             "tb": tb, "mix": mix, "rmat": rd, "brow": brow}
            for c in range(N_CORES)
        ]
        res = _run(_CACHE["fft"], in_maps)

    out = np.empty((B, D), dtype=np.float32)
    for c in range(N_CORES):
        out[c * NT:(c + 1) * NT, :] = res.results[c]["yT"].T
    return out
